# revision 11
# baseline (speedup 1.0000x reference)
"""Trainium2 Bass kernel for nn_AssignAttention (softmax over the query axis).

Math (per batch b):
  q = (query @ Wq)  [N, C] -> heads [N, H, hd]
  k = (key   @ Wk)  [S, C] -> heads [S, H, hd]
  raw[h, n, s] = (q_h @ k_h^T) * hd^-0.5
  attn = softmax(raw, axis=n)                  # normalize over queries, per (h, s)
  attn = attn / max(sum_s attn, 1)             # clamp-normalize over s, per (h, n)
  out[n, h*hd:] = sum_s attn[h, n, s] * key[s, h*hd:(h+1)*hd]
  returns (out, out_style) with out_style == out

Distribution: data-parallel over B=16 across 8 NeuronCores (2 batches/core).

v2 dataflow per core (all matmuls bf16, accumulation f32):
  - key is cast+scattered (f32->bf16) into SBUF as [s-part, t, h, 1+64] with a
    memset 1.0 column per head ("kb65"): the out-matmul rhs [r|v] comes for
    free and its col 0 yields the clamp divisor (no separate div matmuls).
  - bf16 key is bounced to DRAM scratch and read back with the HWDGE xbar
    DMA-transpose to get keyT (no TensorE transposes at all).
  - k/q projections computed in transposed [c_out, s] layout on TensorE.
  - scores[s-part, n-free] per (h, t); heads grouped by row-group parity into
    two [128,1024] PSUM tiles (mixed row-groups in one PSUM bank is a device
    crash).
  - exp: tile A (even heads) = one FD-1024 ScalarE instr (denominators via one
    DVE grouped tensor_reduce); tile B (odd heads) = four FD-256 instrs with
    accum_out (denominator free).  This balances ScalarE vs VectorE.
  - 1/D[s] folded into the tiny rhs: vaug = kb65-slice * r -> [r | v*r].
  - out_acc[n, 65] += e.T @ vaug accumulated over t in PSUM, one logical
    accumulation group per PSUM bank (start only on the bank's first matmul).
"""

import os
import threading
import contextlib

import numpy as np

B, N, S, C, H = 16, 256, 4096, 512, 8
HD = C // H
NCORES = 8
BL = B // NCORES  # batches per core
SCALE = float(HD) ** -0.5

_cache = {}
_lock = threading.Lock()

# head order: even heads (row-group 0) first, then odd (row-group 64)
HORD = [0, 2, 4, 6, 1, 3, 5, 7]


def _g_col(g):
    # 65-wide blocks packed 7 per PSUM bank (65*8 > 512 would cross banks)
    return (g // 7) * 512 + (g % 7) * 65


def _build():
    from contextlib import ExitStack

    import concourse.bass as bass
    import concourse.tile as tile
    from concourse import bacc, mybir
    f32 = mybir.dt.float32
    bf16 = mybir.dt.bfloat16

    nc = bacc.Bacc(
        "TRN2",
        target_bir_lowering=False,
        debug=False,
        enable_asserts=False,
        num_devices=NCORES,
    )
    q_ap = nc.dram_tensor("query", [BL, N, C], f32, kind="ExternalInput").ap()
    k_ap = nc.dram_tensor("key", [BL, S, C], f32, kind="ExternalInput").ap()
    wq_ap = nc.dram_tensor("Wq", [C, C], f32, kind="ExternalInput").ap()
    wk_ap = nc.dram_tensor("Wk", [C, C], f32, kind="ExternalInput").ap()
    out_ap = nc.dram_tensor("out", [BL, N, C], f32, kind="ExternalOutput").ap()
    out2_ap = nc.dram_tensor("out_style", [BL, N, C], f32, kind="ExternalOutput").ap()

    NT = S // 128   # 32 s-tiles
    NCK = C // 128  # c_in chunks
    NM = C // 128   # c_out chunks

    with tile.TileContext(nc) as tc, ExitStack() as ctx:
        const = ctx.enter_context(tc.tile_pool(name="const", bufs=1))
        wq_bf = const.tile([128, NCK * C], bf16)
        wk_bf = const.tile([128, NCK * C], bf16)
        nc.gpsimd.dma_start(
            wq_bf[:].rearrange("p (k c) -> p k c", k=NCK),
            wq_ap.rearrange("(k p) c -> p k c", k=NCK),
        )
        nc.gpsimd.dma_start(
            wk_bf[:].rearrange("p (k c) -> p k c", k=NCK),
            wk_ap.rearrange("(k p) c -> p k c", k=NCK),
        )

        # SBUF pools
        kb_pool = ctx.enter_context(tc.tile_pool(name="kb", bufs=2))
        ktp_pool = ctx.enter_context(tc.tile_pool(name="ktp", bufs=2))
        ktin_pool = ctx.enter_context(tc.tile_pool(name="ktin", bufs=1))
        qpool = ctx.enter_context(tc.tile_pool(name="qpool", bufs=2))
        epool = ctx.enter_context(tc.tile_pool(name="epool", bufs=3))
        spool = ctx.enter_context(tc.tile_pool(name="spool", bufs=3))
        opool = ctx.enter_context(tc.tile_pool(name="opool", bufs=2))
        # DRAM bounce scratch
        dpool = ctx.enter_context(tc.tile_pool(name="dram", bufs=2, space="DRAM"))

        # PSUM pools: kprj 1 + scA 2 + scB 2 + oacc 3 = 8 banks
        kprj_pool = ctx.enter_context(tc.tile_pool(name="kprj", bufs=1, space="PSUM"))
        scA_pool = ctx.enter_context(tc.tile_pool(name="scA", bufs=1, space="PSUM"))
        scB_pool = ctx.enter_context(tc.tile_pool(name="scB", bufs=1, space="PSUM"))
        oacc_pool = ctx.enter_context(tc.tile_pool(name="oacc", bufs=1, space="PSUM"))

        for b in range(BL):
            # ---------- q path: bounce-transpose + projection ----------
            qsc = dpool.tile([N, C], bf16, tag="qsc")
            nc.gpsimd.dma_start(qsc[:], q_ap[b])  # DRAM->DRAM cast f32->bf16
            qt_sb = qpool.tile([128, NCK * N], bf16, tag="qt")
            for ck in range(NCK):
                nc.sync.dma_start(
                    qt_sb[:, ck * N : (ck + 1) * N],
                    qsc[:, ck * 128 : (ck + 1) * 128],
                    transpose=True,
                )
            qtp = qpool.tile([128, NM * N], bf16, tag="qtp")
            for m in range(NM):
                pq = kprj_pool.tile([128, 512], f32, tag="kprj")
                for k in range(NCK):
                    nc.tensor.matmul(
                        pq[:, :N],
                        lhsT=wq_bf[:, k * C + m * 128 : k * C + (m + 1) * 128],
                        rhs=qt_sb[:, k * N : (k + 1) * N],
                        start=(k == 0),
                        stop=(k == NCK - 1),
                    )
                nc.vector.tensor_copy(qtp[:, m * N : (m + 1) * N], pq[:, :N])

            # ---------- k path ----------
            # kb65: [s-part, (t, h, 1+64)] with ones col per head
            kb = kb_pool.tile([128, NT * H * 65], bf16, tag="kb")
            kb4 = kb[:].rearrange("p (t h x) -> p t h x", t=NT, h=H)
            nc.vector.memset(kb4[:, :, :, 0:1], 1.0)
            kin4 = k_ap[b].rearrange("(t p) (h c) -> p t h c", t=NT, h=H)
            for h in range(H):
                nc.gpsimd.dma_start(kb4[:, :, h, 1:65], kin4[:, :, h, :])
            # bounce bf16 key to DRAM, then xbar-transpose-read keyT
            ksc = dpool.tile([S, C], bf16, tag="ksc")
            ksc4 = ksc[:].rearrange("(t p) (h c) -> p t h c", t=NT, h=H)
            for h in range(H):
                nc.sync.dma_start(ksc4[:, :, h, :], kb4[:, :, h, 1:65])
            ktin = ktin_pool.tile([128, NCK * S], bf16, tag="ktin")
            for ck in range(NCK):
                nc.sync.dma_start(
                    ktin[:, ck * S : (ck + 1) * S],
                    ksc[:, ck * 128 : (ck + 1) * 128],
                    transpose=True,
                )
            # k projection, transposed output kT [c_out(part by chunk m), s]
            ktp = ktp_pool.tile([128, NM * S], bf16, tag="ktp")
            for m in range(NM):
                for sb in range(S // 512):
                    pk = kprj_pool.tile([128, 512], f32, tag="kprj")
                    for k in range(NCK):
                        nc.tensor.matmul(
                            pk[:],
                            lhsT=wk_bf[:, k * C + m * 128 : k * C + (m + 1) * 128],
                            rhs=ktin[:, k * S + sb * 512 : k * S + (sb + 1) * 512],
                            start=(k == 0),
                            stop=(k == NCK - 1),
                        )
                    nc.vector.tensor_copy(
                        ktp[:, m * S + sb * 512 : m * S + (sb + 1) * 512], pk[:]
                    )

            # ---------- attention ----------
            oacc = oacc_pool.tile([128, 1536], f32, tag="oacc")
            for t in range(NT):
                # scores: tile A = even heads (row group 0), B = odd (rg 64)
                scA = scA_pool.tile([128, 1024], f32, tag="scA")
                scB = scB_pool.tile([128, 1024], f32, tag="scB")
                for i in range(4):
                    for sc, h in ((scA, HORD[i]), (scB, HORD[4 + i])):
                        m, hp = h // 2, (h % 2) * 64
                        nc.tensor.matmul(
                            sc[:, i * N : (i + 1) * N],
                            lhsT=ktp[
                                hp : hp + 64, m * S + t * 128 : m * S + t * 128 + 128
                            ],
                            rhs=qtp[hp : hp + 64, m * N : (m + 1) * N],
                            start=True,
                            stop=True,
                        )
                # exp: A in one FD-1024 instr; B as 4 FD-256 instrs with accum
                et = epool.tile([128, 2 * 1024], bf16, tag="et")
                den = spool.tile([128, 8], f32, tag="den")
                nc.scalar.activation(
                    et[:, 0:1024],
                    scA[:],
                    mybir.ActivationFunctionType.Exp,
                    scale=SCALE,
                )
                for i in range(4):
                    nc.scalar.activation(
                        et[:, 1024 + i * N : 1024 + (i + 1) * N],
                        scB[:, i * N : (i + 1) * N],
                        mybir.ActivationFunctionType.Exp,
                        scale=SCALE,
                        accum_out=den[:, 4 + i : 5 + i],
                    )
                # denominators for A heads: one grouped reduce
                nc.vector.tensor_reduce(
                    den[:, 0:4],
                    et[:, 0:1024].rearrange("p (g n) -> p g n", g=4),
                    mybir.AxisListType.X,
                    mybir.AluOpType.add,
                )
                rt = spool.tile([128, 8], f32, tag="rt")
                nc.vector.reciprocal(rt[:], den[:])
                # vaug[idx] = [r | v*r] from the ones-embedded kb slice
                vaug = spool.tile([128, 8 * 65], bf16, tag="vaug")
                for idx in range(8):
                    h = HORD[idx]
                    nc.vector.tensor_scalar_mul(
                        vaug[:, idx * 65 : (idx + 1) * 65],
                        kb[:, (t * H + h) * 65 : (t * H + h) * 65 + 65],
                        rt[:, idx : idx + 1],
                    )
                # out matmuls: one accumulation group per PSUM bank
                crit = (
                    tc.tile_critical()
                    if (t == 0 or t == NT - 1)
                    else contextlib.nullcontext()
                )
                with crit:
                    for idx in range(8):
                        for ncn in range(2):
                            g = idx * 2 + ncn
                            nc.tensor.matmul(
                                oacc[:, _g_col(g) : _g_col(g) + 65],
                                lhsT=et[
                                    :, idx * N + ncn * 128 : idx * N + ncn * 128 + 128
                                ],
                                rhs=vaug[:, idx * 65 : (idx + 1) * 65],
                                start=(t == 0 and g in (0, 7, 14)),
                                stop=(t == NT - 1 and g in (6, 13, 15)),
                                skip_group_check=True,
                            )

            # ---------- epilogue ----------
            divs = spool.tile([128, 16], f32, tag="divs")
            for g in range(16):
                nc.vector.tensor_copy(
                    divs[:, g : g + 1], oacc[:, _g_col(g) : _g_col(g) + 1]
                )
            dm = spool.tile([128, 16], f32, tag="dm")
            nc.vector.tensor_scalar_max(dm[:], divs[:], 1.0)
            rdiv = spool.tile([128, 16], f32, tag="rdiv")
            nc.vector.reciprocal(rdiv[:], dm[:])
            for ncn in range(2):
                osb = opool.tile([128, C], f32, tag="osb")
                for idx in range(8):
                    h = HORD[idx]
                    g = idx * 2 + ncn
                    nc.vector.tensor_scalar_mul(
                        osb[:, h * HD : (h + 1) * HD],
                        oacc[:, _g_col(g) + 1 : _g_col(g) + 65],
                        rdiv[:, g : g + 1],
                    )
                nc.sync.dma_start(out_ap[b, ncn * 128 : (ncn + 1) * 128, :], osb[:])
                nc.sync.dma_start(out2_ap[b, ncn * 128 : (ncn + 1) * 128, :], osb[:])

    nc.compile()
    return nc


def _get_nc():
    with _lock:
        if "nc" not in _cache:
            _cache["nc"] = _build()
        return _cache["nc"]


def kernel(query, key, Wq, Wk):
    from concourse.bass_utils import run_bass_kernel_spmd

    nc = _get_nc()
    query = np.ascontiguousarray(query, dtype=np.float32)
    key = np.ascontiguousarray(key, dtype=np.float32)
    Wq = np.ascontiguousarray(Wq, dtype=np.float32)
    Wk = np.ascontiguousarray(Wk, dtype=np.float32)
    in_maps = [
        {
            "query": query[c * BL : (c + 1) * BL],
            "key": key[c * BL : (c + 1) * BL],
            "Wq": Wq,
            "Wk": Wk,
        }
        for c in range(NCORES)
    ]
    res = run_bass_kernel_spmd(nc, in_maps, core_ids=list(range(NCORES)))
    out = np.concatenate([r["out"] for r in res.results], axis=0)
    out_style = np.concatenate([r["out_style"] for r in res.results], axis=0)
    return out, out_style


# revision 12
# speedup vs baseline: 1.0234x; 1.0234x over previous
"""Trainium2 Bass kernel for nn_AssignAttention (softmax over the query axis).

Math (per batch b):
  q = (query @ Wq)  [N, C] -> heads [N, H, hd]
  k = (key   @ Wk)  [S, C] -> heads [S, H, hd]
  raw[h, n, s] = (q_h @ k_h^T) * hd^-0.5
  attn = softmax(raw, axis=n)                  # normalize over queries, per (h, s)
  attn = attn / max(sum_s attn, 1)             # clamp-normalize over s, per (h, n)
  out[n, h*hd:] = sum_s attn[h, n, s] * key[s, h*hd:(h+1)*hd]
  returns (out, out_style) with out_style == out

Distribution: data-parallel over B=16 across 8 NeuronCores (2 batches/core).

v2 dataflow per core (all matmuls bf16, accumulation f32):
  - key is cast+scattered (f32->bf16) into SBUF as [s-part, t, h, 1+64] with a
    memset 1.0 column per head ("kb65"): the out-matmul rhs [r|v] comes for
    free and its col 0 yields the clamp divisor (no separate div matmuls).
  - bf16 key is bounced to DRAM scratch and read back with the HWDGE xbar
    DMA-transpose to get keyT (no TensorE transposes at all).
  - k/q projections computed in transposed [c_out, s] layout on TensorE.
  - scores[s-part, n-free] per (h, t); heads grouped by row-group parity into
    two [128,1024] PSUM tiles (mixed row-groups in one PSUM bank is a device
    crash).
  - exp: tile A (even heads) = one FD-1024 ScalarE instr (denominators via one
    DVE grouped tensor_reduce); tile B (odd heads) = four FD-256 instrs with
    accum_out (denominator free).  This balances ScalarE vs VectorE.
  - 1/D[s] folded into the tiny rhs: vaug = kb65-slice * r -> [r | v*r].
  - out_acc[n, 65] += e.T @ vaug accumulated over t in PSUM, one logical
    accumulation group per PSUM bank (start only on the bank's first matmul).
"""

import os
import threading
import contextlib

import numpy as np

B, N, S, C, H = 16, 256, 4096, 512, 8
HD = C // H
NCORES = 8
BL = B // NCORES  # batches per core
SCALE = float(HD) ** -0.5

_cache = {}
_lock = threading.Lock()

# head order: even heads (row-group 0) first, then odd (row-group 64)
HORD = [0, 2, 4, 6, 1, 3, 5, 7]


def _g_col(g):
    # 65-wide blocks packed 7 per PSUM bank (65*8 > 512 would cross banks)
    return (g // 7) * 512 + (g % 7) * 65


def _build():
    from contextlib import ExitStack

    import concourse.bass as bass
    import concourse.tile as tile
    from concourse import bacc, mybir
    f32 = mybir.dt.float32
    bf16 = mybir.dt.bfloat16

    nc = bacc.Bacc(
        "TRN2",
        target_bir_lowering=False,
        debug=False,
        enable_asserts=False,
        num_devices=NCORES,
    )
    q_ap = nc.dram_tensor("query", [BL, N, C], f32, kind="ExternalInput").ap()
    k_ap = nc.dram_tensor("key", [BL, S, C], f32, kind="ExternalInput").ap()
    wq_ap = nc.dram_tensor("Wq", [C, C], f32, kind="ExternalInput").ap()
    wk_ap = nc.dram_tensor("Wk", [C, C], f32, kind="ExternalInput").ap()
    out_ap = nc.dram_tensor("out", [BL, N, C], f32, kind="ExternalOutput").ap()
    out2_ap = nc.dram_tensor("out_style", [BL, N, C], f32, kind="ExternalOutput").ap()

    NT = S // 128   # 32 s-tiles
    NCK = C // 128  # c_in chunks
    NM = C // 128   # c_out chunks

    with tile.TileContext(nc) as tc, ExitStack() as ctx:
        const = ctx.enter_context(tc.tile_pool(name="const", bufs=1))
        wq_bf = const.tile([128, NCK * C], bf16)
        wk_bf = const.tile([128, NCK * C], bf16)
        nc.gpsimd.dma_start(
            wq_bf[:].rearrange("p (k c) -> p k c", k=NCK),
            wq_ap.rearrange("(k p) c -> p k c", k=NCK),
        )
        nc.gpsimd.dma_start(
            wk_bf[:].rearrange("p (k c) -> p k c", k=NCK),
            wk_ap.rearrange("(k p) c -> p k c", k=NCK),
        )

        # SBUF pools
        kb_pool = ctx.enter_context(tc.tile_pool(name="kb", bufs=2))
        ktp_pool = ctx.enter_context(tc.tile_pool(name="ktp", bufs=2))
        ktin_pool = ctx.enter_context(tc.tile_pool(name="ktin", bufs=1))
        qpool = ctx.enter_context(tc.tile_pool(name="qpool", bufs=2))
        epool = ctx.enter_context(tc.tile_pool(name="epool", bufs=3))
        spool = ctx.enter_context(tc.tile_pool(name="spool", bufs=3))
        opool = ctx.enter_context(tc.tile_pool(name="opool", bufs=2))
        # DRAM bounce scratch
        dpool = ctx.enter_context(tc.tile_pool(name="dram", bufs=2, space="DRAM"))

        # PSUM pools: kprj 1 + scA 2 + scB 2 + oacc 3 = 8 banks
        kprj_pool = ctx.enter_context(tc.tile_pool(name="kprj", bufs=1, space="PSUM"))
        scA_pool = ctx.enter_context(tc.tile_pool(name="scA", bufs=1, space="PSUM"))
        scB_pool = ctx.enter_context(tc.tile_pool(name="scB", bufs=1, space="PSUM"))
        oacc_pool = ctx.enter_context(tc.tile_pool(name="oacc", bufs=1, space="PSUM"))

        for b in range(BL):
            # ---------- q path: bounce-transpose + projection ----------
            qsc = dpool.tile([N, C], bf16, tag="qsc")
            nc.gpsimd.dma_start(qsc[:], q_ap[b])  # DRAM->DRAM cast f32->bf16
            qt_sb = qpool.tile([128, NCK * N], bf16, tag="qt")
            for ck in range(NCK):
                nc.sync.dma_start(
                    qt_sb[:, ck * N : (ck + 1) * N],
                    qsc[:, ck * 128 : (ck + 1) * 128],
                    transpose=True,
                )
            qtp = qpool.tile([128, NM * N], bf16, tag="qtp")
            for m in range(NM):
                pq = kprj_pool.tile([128, 512], f32, tag="kprj")
                for k in range(NCK):
                    nc.tensor.matmul(
                        pq[:, :N],
                        lhsT=wq_bf[:, k * C + m * 128 : k * C + (m + 1) * 128],
                        rhs=qt_sb[:, k * N : (k + 1) * N],
                        start=(k == 0),
                        stop=(k == NCK - 1),
                    )
                nc.vector.tensor_copy(qtp[:, m * N : (m + 1) * N], pq[:, :N])

            # ---------- k path ----------
            # kb65: [s-part, (t, h, 1+64)] with ones col per head
            kb = kb_pool.tile([128, NT * H * 65], bf16, tag="kb")
            kb4 = kb[:].rearrange("p (t h x) -> p t h x", t=NT, h=H)
            nc.vector.memset(kb4[:, :, :, 0:1], 1.0)
            kin4 = k_ap[b].rearrange("(t p) (h c) -> p t h c", t=NT, h=H)
            for h in range(H):
                nc.gpsimd.dma_start(kb4[:, :, h, 1:65], kin4[:, :, h, :])
            # bf16 scratch copy of key straight from DRAM (cast during DMA),
            # chunked by 512-row blocks; then xbar-transpose-read keyT per
            # block so the k-projection pipelines behind the DMA chain.
            ksc = dpool.tile([S, C], bf16, tag="ksc")
            for sb in range(S // 512):
                nc.gpsimd.dma_start(
                    ksc[sb * 512 : (sb + 1) * 512, :],
                    k_ap[b, sb * 512 : (sb + 1) * 512, :],
                )
            ktin = ktin_pool.tile([128, NCK * S], bf16, tag="ktin")
            ktp = ktp_pool.tile([128, NM * S], bf16, tag="ktp")
            for sb in range(S // 512):
                for ck in range(NCK):
                    nc.sync.dma_start(
                        ktin[:, ck * S + sb * 512 : ck * S + (sb + 1) * 512],
                        ksc[sb * 512 : (sb + 1) * 512, ck * 128 : (ck + 1) * 128],
                        transpose=True,
                    )
                for m in range(NM):
                    pk = kprj_pool.tile([128, 512], f32, tag="kprj")
                    for k in range(NCK):
                        nc.tensor.matmul(
                            pk[:],
                            lhsT=wk_bf[:, k * C + m * 128 : k * C + (m + 1) * 128],
                            rhs=ktin[:, k * S + sb * 512 : k * S + (sb + 1) * 512],
                            start=(k == 0),
                            stop=(k == NCK - 1),
                        )
                    nc.vector.tensor_copy(
                        ktp[:, m * S + sb * 512 : m * S + (sb + 1) * 512], pk[:]
                    )

            # ---------- attention ----------
            oacc = oacc_pool.tile([128, 1536], f32, tag="oacc")
            for t in range(NT):
                # scores: tile A = even heads (row group 0), B = odd (rg 64)
                scA = scA_pool.tile([128, 1024], f32, tag="scA")
                scB = scB_pool.tile([128, 1024], f32, tag="scB")
                for i in range(4):
                    for sc, h in ((scA, HORD[i]), (scB, HORD[4 + i])):
                        m, hp = h // 2, (h % 2) * 64
                        nc.tensor.matmul(
                            sc[:, i * N : (i + 1) * N],
                            lhsT=ktp[
                                hp : hp + 64, m * S + t * 128 : m * S + t * 128 + 128
                            ],
                            rhs=qtp[hp : hp + 64, m * N : (m + 1) * N],
                            start=True,
                            stop=True,
                        )
                # exp: A in one FD-1024 instr; B as 4 FD-256 instrs with accum
                et = epool.tile([128, 2 * 1024], bf16, tag="et")
                den = spool.tile([128, 8], f32, tag="den")
                nc.scalar.activation(
                    et[:, 0:1024],
                    scA[:],
                    mybir.ActivationFunctionType.Exp,
                    scale=SCALE,
                )
                for i in range(4):
                    nc.scalar.activation(
                        et[:, 1024 + i * N : 1024 + (i + 1) * N],
                        scB[:, i * N : (i + 1) * N],
                        mybir.ActivationFunctionType.Exp,
                        scale=SCALE,
                        accum_out=den[:, 4 + i : 5 + i],
                    )
                # denominators for A heads: one grouped reduce
                nc.vector.tensor_reduce(
                    den[:, 0:4],
                    et[:, 0:1024].rearrange("p (g n) -> p g n", g=4),
                    mybir.AxisListType.X,
                    mybir.AluOpType.add,
                )
                rt = spool.tile([128, 8], f32, tag="rt")
                nc.vector.reciprocal(rt[:], den[:])
                # vaug[idx] = [r | v*r] from the ones-embedded kb slice
                vaug = spool.tile([128, 8 * 65], bf16, tag="vaug")
                for idx in range(8):
                    h = HORD[idx]
                    nc.vector.tensor_scalar_mul(
                        vaug[:, idx * 65 : (idx + 1) * 65],
                        kb[:, (t * H + h) * 65 : (t * H + h) * 65 + 65],
                        rt[:, idx : idx + 1],
                    )
                # out matmuls: one accumulation group per PSUM bank
                crit = (
                    tc.tile_critical()
                    if (t == 0 or t == NT - 1)
                    else contextlib.nullcontext()
                )
                with crit:
                    for idx in range(8):
                        for ncn in range(2):
                            g = idx * 2 + ncn
                            nc.tensor.matmul(
                                oacc[:, _g_col(g) : _g_col(g) + 65],
                                lhsT=et[
                                    :, idx * N + ncn * 128 : idx * N + ncn * 128 + 128
                                ],
                                rhs=vaug[:, idx * 65 : (idx + 1) * 65],
                                start=(t == 0 and g in (0, 7, 14)),
                                stop=(t == NT - 1 and g in (6, 13, 15)),
                                skip_group_check=True,
                            )

            # ---------- epilogue ----------
            divs = spool.tile([128, 16], f32, tag="divs")
            for g in range(16):
                nc.vector.tensor_copy(
                    divs[:, g : g + 1], oacc[:, _g_col(g) : _g_col(g) + 1]
                )
            dm = spool.tile([128, 16], f32, tag="dm")
            nc.vector.tensor_scalar_max(dm[:], divs[:], 1.0)
            rdiv = spool.tile([128, 16], f32, tag="rdiv")
            nc.vector.reciprocal(rdiv[:], dm[:])
            for ncn in range(2):
                osb = opool.tile([128, C], f32, tag="osb")
                for idx in range(8):
                    h = HORD[idx]
                    g = idx * 2 + ncn
                    nc.vector.tensor_scalar_mul(
                        osb[:, h * HD : (h + 1) * HD],
                        oacc[:, _g_col(g) + 1 : _g_col(g) + 65],
                        rdiv[:, g : g + 1],
                    )
                nc.sync.dma_start(out_ap[b, ncn * 128 : (ncn + 1) * 128, :], osb[:])
                nc.sync.dma_start(out2_ap[b, ncn * 128 : (ncn + 1) * 128, :], osb[:])

    nc.compile()
    return nc


def _get_nc():
    with _lock:
        if "nc" not in _cache:
            _cache["nc"] = _build()
        return _cache["nc"]


def kernel(query, key, Wq, Wk):
    from concourse.bass_utils import run_bass_kernel_spmd

    nc = _get_nc()
    query = np.ascontiguousarray(query, dtype=np.float32)
    key = np.ascontiguousarray(key, dtype=np.float32)
    Wq = np.ascontiguousarray(Wq, dtype=np.float32)
    Wk = np.ascontiguousarray(Wk, dtype=np.float32)
    in_maps = [
        {
            "query": query[c * BL : (c + 1) * BL],
            "key": key[c * BL : (c + 1) * BL],
            "Wq": Wq,
            "Wk": Wk,
        }
        for c in range(NCORES)
    ]
    res = run_bass_kernel_spmd(nc, in_maps, core_ids=list(range(NCORES)))
    out = np.concatenate([r["out"] for r in res.results], axis=0)
    out_style = np.concatenate([r["out_style"] for r in res.results], axis=0)
    return out, out_style


# revision 13
# speedup vs baseline: 1.0533x; 1.0292x over previous
"""Trainium2 Bass kernel for nn_AssignAttention (softmax over the query axis).

Math (per batch b):
  q = (query @ Wq)  [N, C] -> heads [N, H, hd]
  k = (key   @ Wk)  [S, C] -> heads [S, H, hd]
  raw[h, n, s] = (q_h @ k_h^T) * hd^-0.5
  attn = softmax(raw, axis=n)                  # normalize over queries, per (h, s)
  attn = attn / max(sum_s attn, 1)             # clamp-normalize over s, per (h, n)
  out[n, h*hd:] = sum_s attn[h, n, s] * key[s, h*hd:(h+1)*hd]
  returns (out, out_style) with out_style == out

Distribution: data-parallel over B=16 across 8 NeuronCores (2 batches/core).

v2 dataflow per core (all matmuls bf16, accumulation f32):
  - key is cast+scattered (f32->bf16) into SBUF as [s-part, t, h, 1+64] with a
    memset 1.0 column per head ("kb65"): the out-matmul rhs [r|v] comes for
    free and its col 0 yields the clamp divisor (no separate div matmuls).
  - bf16 key is bounced to DRAM scratch and read back with the HWDGE xbar
    DMA-transpose to get keyT (no TensorE transposes at all).
  - k/q projections computed in transposed [c_out, s] layout on TensorE.
  - scores[s-part, n-free] per (h, t); heads grouped by row-group parity into
    two [128,1024] PSUM tiles (mixed row-groups in one PSUM bank is a device
    crash).
  - exp: tile A (even heads) = one FD-1024 ScalarE instr (denominators via one
    DVE grouped tensor_reduce); tile B (odd heads) = four FD-256 instrs with
    accum_out (denominator free).  This balances ScalarE vs VectorE.
  - 1/D[s] folded into the tiny rhs: vaug = kb65-slice * r -> [r | v*r].
  - out_acc[n, 65] += e.T @ vaug accumulated over t in PSUM, one logical
    accumulation group per PSUM bank (start only on the bank's first matmul).
"""

import os
import threading
import contextlib

import numpy as np

B, N, S, C, H = 16, 256, 4096, 512, 8
HD = C // H
NCORES = 8
BL = B // NCORES  # batches per core
SCALE = float(HD) ** -0.5

_cache = {}
_lock = threading.Lock()

# head order: even heads (row-group 0) first, then odd (row-group 64)
HORD = [0, 2, 4, 6, 1, 3, 5, 7]


def _g_col(g):
    # 65-wide blocks packed 7 per PSUM bank (65*8 > 512 would cross banks)
    return (g // 7) * 512 + (g % 7) * 65


def _build():
    from contextlib import ExitStack

    import concourse.bass as bass
    import concourse.tile as tile
    from concourse import bacc, mybir
    f32 = mybir.dt.float32
    bf16 = mybir.dt.bfloat16

    nc = bacc.Bacc(
        "TRN2",
        target_bir_lowering=False,
        debug=False,
        enable_asserts=False,
        num_devices=NCORES,
    )
    q_ap = nc.dram_tensor("query", [BL, N, C], f32, kind="ExternalInput").ap()
    k_ap = nc.dram_tensor("key", [BL, S, C], f32, kind="ExternalInput").ap()
    wq_ap = nc.dram_tensor("Wq", [C, C], f32, kind="ExternalInput").ap()
    wk_ap = nc.dram_tensor("Wk", [C, C], f32, kind="ExternalInput").ap()
    out_ap = nc.dram_tensor("out", [BL, N, C], f32, kind="ExternalOutput").ap()
    out2_ap = nc.dram_tensor("out_style", [BL, N, C], f32, kind="ExternalOutput").ap()

    NT = S // 128   # 32 s-tiles
    NCK = C // 128  # c_in chunks
    NM = C // 128   # c_out chunks

    with tile.TileContext(nc) as tc, ExitStack() as ctx:
        const = ctx.enter_context(tc.tile_pool(name="const", bufs=1))
        wq_bf = const.tile([128, NCK * C], bf16)
        wk_bf = const.tile([128, NCK * C], bf16)
        nc.gpsimd.dma_start(
            wq_bf[:].rearrange("p (k c) -> p k c", k=NCK),
            wq_ap.rearrange("(k p) c -> p k c", k=NCK),
        )
        nc.gpsimd.dma_start(
            wk_bf[:].rearrange("p (k c) -> p k c", k=NCK),
            wk_ap.rearrange("(k p) c -> p k c", k=NCK),
        )

        # SBUF pools
        kb_pool = ctx.enter_context(tc.tile_pool(name="kb", bufs=2))
        ktp_pool = ctx.enter_context(tc.tile_pool(name="ktp", bufs=2))
        ktin_pool = ctx.enter_context(tc.tile_pool(name="ktin", bufs=1))
        qpool = ctx.enter_context(tc.tile_pool(name="qpool", bufs=2))
        epool = ctx.enter_context(tc.tile_pool(name="epool", bufs=3))
        spool = ctx.enter_context(tc.tile_pool(name="spool", bufs=3))
        opool = ctx.enter_context(tc.tile_pool(name="opool", bufs=2))
        # DRAM bounce scratch
        dpool = ctx.enter_context(tc.tile_pool(name="dram", bufs=2, space="DRAM"))

        # PSUM pools: kprj 1 + scA 2 + scB 2 + oacc 3 = 8 banks
        kprj_pool = ctx.enter_context(tc.tile_pool(name="kprj", bufs=1, space="PSUM"))
        scA_pool = ctx.enter_context(tc.tile_pool(name="scA", bufs=1, space="PSUM"))
        scB_pool = ctx.enter_context(tc.tile_pool(name="scB", bufs=1, space="PSUM"))
        oacc_pool = ctx.enter_context(tc.tile_pool(name="oacc", bufs=1, space="PSUM"))

        for b in range(BL):
            # ---------- q path: bounce-transpose + projection ----------
            qsc = dpool.tile([N, C], bf16, tag="qsc")
            nc.gpsimd.dma_start(qsc[:], q_ap[b])  # DRAM->DRAM cast f32->bf16
            qt_sb = qpool.tile([128, NCK * N], bf16, tag="qt")
            for ck in range(NCK):
                nc.sync.dma_start(
                    qt_sb[:, ck * N : (ck + 1) * N],
                    qsc[:, ck * 128 : (ck + 1) * 128],
                    transpose=True,
                )
            qtp = qpool.tile([128, NM * N], bf16, tag="qtp")
            for m in range(NM):
                pq = kprj_pool.tile([128, 512], f32, tag="kprj")
                for k in range(NCK):
                    nc.tensor.matmul(
                        pq[:, :N],
                        lhsT=wq_bf[:, k * C + m * 128 : k * C + (m + 1) * 128],
                        rhs=qt_sb[:, k * N : (k + 1) * N],
                        start=(k == 0),
                        stop=(k == NCK - 1),
                    )
                nc.vector.tensor_copy(qtp[:, m * N : (m + 1) * N], pq[:, :N])

            # ---------- k path ----------
            # kb65: [s-part, (t, h, 1+64)] with ones col per head
            kb = kb_pool.tile([128, NT * H * 65], bf16, tag="kb")
            kb4 = kb[:].rearrange("p (t h x) -> p t h x", t=NT, h=H)
            nc.vector.memset(kb4[:, :, :, 0:1], 1.0)
            # (filled per s-block below, from the bf16 scratch)
            # bf16 scratch copy of key straight from DRAM (cast during DMA),
            # chunked by 512-row blocks; then xbar-transpose-read keyT per
            # block so the k-projection pipelines behind the DMA chain.
            ksc = dpool.tile([S, C], bf16, tag="ksc")
            for sb in range(S // 512):
                nc.gpsimd.dma_start(
                    ksc[sb * 512 : (sb + 1) * 512, :],
                    k_ap[b, sb * 512 : (sb + 1) * 512, :],
                )
            ktin = ktin_pool.tile([128, NCK * S], bf16, tag="ktin")
            ktp = ktp_pool.tile([128, NM * S], bf16, tag="ktp")
            for sb in range(S // 512):
                for h in range(H):
                    nc.sync.dma_start(
                        kb4[:, 4 * sb : 4 * sb + 4, h, 1:65],
                        ksc[sb * 512 : (sb + 1) * 512, h * HD : (h + 1) * HD]
                        .rearrange("(t p) c -> p t c", t=4),
                    )
                for ck in range(NCK):
                    nc.sync.dma_start(
                        ktin[:, ck * S + sb * 512 : ck * S + (sb + 1) * 512],
                        ksc[sb * 512 : (sb + 1) * 512, ck * 128 : (ck + 1) * 128],
                        transpose=True,
                    )
                for m in range(NM):
                    pk = kprj_pool.tile([128, 512], f32, tag="kprj")
                    for k in range(NCK):
                        nc.tensor.matmul(
                            pk[:],
                            lhsT=wk_bf[:, k * C + m * 128 : k * C + (m + 1) * 128],
                            rhs=ktin[:, k * S + sb * 512 : k * S + (sb + 1) * 512],
                            start=(k == 0),
                            stop=(k == NCK - 1),
                        )
                    nc.vector.tensor_copy(
                        ktp[:, m * S + sb * 512 : m * S + (sb + 1) * 512], pk[:]
                    )

            # ---------- attention ----------
            oacc = oacc_pool.tile([128, 1536], f32, tag="oacc")
            for t in range(NT):
                # scores: tile A = even heads (row group 0), B = odd (rg 64)
                scA = scA_pool.tile([128, 1024], f32, tag="scA")
                scB = scB_pool.tile([128, 1024], f32, tag="scB")
                for i in range(4):
                    for sc, h in ((scA, HORD[i]), (scB, HORD[4 + i])):
                        m, hp = h // 2, (h % 2) * 64
                        nc.tensor.matmul(
                            sc[:, i * N : (i + 1) * N],
                            lhsT=ktp[
                                hp : hp + 64, m * S + t * 128 : m * S + t * 128 + 128
                            ],
                            rhs=qtp[hp : hp + 64, m * N : (m + 1) * N],
                            start=True,
                            stop=True,
                        )
                # exp: A in one FD-1024 instr; B as 4 FD-256 instrs with accum
                et = epool.tile([128, 2 * 1024], bf16, tag="et")
                den = spool.tile([128, 8], f32, tag="den")
                nc.scalar.activation(
                    et[:, 0:1024],
                    scA[:],
                    mybir.ActivationFunctionType.Exp,
                    scale=SCALE,
                )
                for i in range(4):
                    nc.scalar.activation(
                        et[:, 1024 + i * N : 1024 + (i + 1) * N],
                        scB[:, i * N : (i + 1) * N],
                        mybir.ActivationFunctionType.Exp,
                        scale=SCALE,
                        accum_out=den[:, 4 + i : 5 + i],
                    )
                # denominators for A heads: one grouped reduce
                nc.vector.tensor_reduce(
                    den[:, 0:4],
                    et[:, 0:1024].rearrange("p (g n) -> p g n", g=4),
                    mybir.AxisListType.X,
                    mybir.AluOpType.add,
                )
                rt = spool.tile([128, 8], f32, tag="rt")
                nc.vector.reciprocal(rt[:], den[:])
                # vaug[idx] = [r | v*r] from the ones-embedded kb slice
                vaug = spool.tile([128, 8 * 65], bf16, tag="vaug")
                for idx in range(8):
                    h = HORD[idx]
                    nc.vector.tensor_scalar_mul(
                        vaug[:, idx * 65 : (idx + 1) * 65],
                        kb[:, (t * H + h) * 65 : (t * H + h) * 65 + 65],
                        rt[:, idx : idx + 1],
                    )
                # out matmuls: one accumulation group per PSUM bank
                crit = (
                    tc.tile_critical()
                    if (t == 0 or t == NT - 1)
                    else contextlib.nullcontext()
                )
                with crit:
                    for idx in range(8):
                        for ncn in range(2):
                            g = idx * 2 + ncn
                            nc.tensor.matmul(
                                oacc[:, _g_col(g) : _g_col(g) + 65],
                                lhsT=et[
                                    :, idx * N + ncn * 128 : idx * N + ncn * 128 + 128
                                ],
                                rhs=vaug[:, idx * 65 : (idx + 1) * 65],
                                start=(t == 0 and g in (0, 7, 14)),
                                stop=(t == NT - 1 and g in (6, 13, 15)),
                                skip_group_check=True,
                            )

            # ---------- epilogue ----------
            dm = spool.tile([128, 16], f32, tag="dm")
            nc.vector.tensor_scalar_max(
                dm[:, 0:7],
                oacc[:, 0:455].rearrange("p (g x) -> p g x", g=7)[:, :, 0:1],
                1.0,
            )
            nc.vector.tensor_scalar_max(
                dm[:, 7:14],
                oacc[:, 512:967].rearrange("p (g x) -> p g x", g=7)[:, :, 0:1],
                1.0,
            )
            nc.vector.tensor_scalar_max(
                dm[:, 14:16],
                oacc[:, 1024:1154].rearrange("p (g x) -> p g x", g=2)[:, :, 0:1],
                1.0,
            )
            rdiv = spool.tile([128, 16], f32, tag="rdiv")
            nc.vector.reciprocal(rdiv[:], dm[:])
            for ncn in range(2):
                osb = opool.tile([128, C], f32, tag="osb")
                for idx in range(8):
                    h = HORD[idx]
                    g = idx * 2 + ncn
                    nc.vector.tensor_scalar_mul(
                        osb[:, h * HD : (h + 1) * HD],
                        oacc[:, _g_col(g) + 1 : _g_col(g) + 65],
                        rdiv[:, g : g + 1],
                    )
                nc.sync.dma_start(out_ap[b, ncn * 128 : (ncn + 1) * 128, :], osb[:])
                nc.sync.dma_start(out2_ap[b, ncn * 128 : (ncn + 1) * 128, :], osb[:])

    nc.compile()
    return nc


def _get_nc():
    with _lock:
        if "nc" not in _cache:
            _cache["nc"] = _build()
        return _cache["nc"]


def kernel(query, key, Wq, Wk):
    from concourse.bass_utils import run_bass_kernel_spmd

    nc = _get_nc()
    query = np.ascontiguousarray(query, dtype=np.float32)
    key = np.ascontiguousarray(key, dtype=np.float32)
    Wq = np.ascontiguousarray(Wq, dtype=np.float32)
    Wk = np.ascontiguousarray(Wk, dtype=np.float32)
    in_maps = [
        {
            "query": query[c * BL : (c + 1) * BL],
            "key": key[c * BL : (c + 1) * BL],
            "Wq": Wq,
            "Wk": Wk,
        }
        for c in range(NCORES)
    ]
    res = run_bass_kernel_spmd(nc, in_maps, core_ids=list(range(NCORES)))
    out = np.concatenate([r["out"] for r in res.results], axis=0)
    out_style = np.concatenate([r["out_style"] for r in res.results], axis=0)
    return out, out_style


# revision 14
# speedup vs baseline: 1.0999x; 1.0442x over previous
"""Trainium2 Bass kernel for nn_AssignAttention (softmax over the query axis).

Math (per batch b):
  q = (query @ Wq)  [N, C] -> heads [N, H, hd]
  k = (key   @ Wk)  [S, C] -> heads [S, H, hd]
  raw[h, n, s] = (q_h @ k_h^T) * hd^-0.5
  attn = softmax(raw, axis=n)                  # normalize over queries, per (h, s)
  attn = attn / max(sum_s attn, 1)             # clamp-normalize over s, per (h, n)
  out[n, h*hd:] = sum_s attn[h, n, s] * key[s, h*hd:(h+1)*hd]
  returns (out, out_style) with out_style == out

Distribution: data-parallel over B=16 across 8 NeuronCores (2 batches/core).

v2 dataflow per core (all matmuls bf16, accumulation f32):
  - key is cast+scattered (f32->bf16) into SBUF as [s-part, t, h, 1+64] with a
    memset 1.0 column per head ("kb65"): the out-matmul rhs [r|v] comes for
    free and its col 0 yields the clamp divisor (no separate div matmuls).
  - bf16 key is bounced to DRAM scratch and read back with the HWDGE xbar
    DMA-transpose to get keyT (no TensorE transposes at all).
  - k/q projections computed in transposed [c_out, s] layout on TensorE.
  - scores[s-part, n-free] per (h, t); heads grouped by row-group parity into
    two [128,1024] PSUM tiles (mixed row-groups in one PSUM bank is a device
    crash).
  - exp: tile A (even heads) = one FD-1024 ScalarE instr (denominators via one
    DVE grouped tensor_reduce); tile B (odd heads) = four FD-256 instrs with
    accum_out (denominator free).  This balances ScalarE vs VectorE.
  - 1/D[s] folded into the tiny rhs: vaug = kb65-slice * r -> [r | v*r].
  - out_acc[n, 65] += e.T @ vaug accumulated over t in PSUM, one logical
    accumulation group per PSUM bank (start only on the bank's first matmul).
"""

import os
import threading
import contextlib

import numpy as np

B, N, S, C, H = 16, 256, 4096, 512, 8
HD = C // H
NCORES = 8
BL = B // NCORES  # batches per core
SCALE = float(HD) ** -0.5

_cache = {}
_lock = threading.Lock()

# head order: even heads (row-group 0) first, then odd (row-group 64)
HORD = [0, 2, 4, 6, 1, 3, 5, 7]


def _g_col(g):
    # 65-wide blocks packed 7 per PSUM bank (65*8 > 512 would cross banks)
    return (g // 7) * 512 + (g % 7) * 65


def _build():
    from contextlib import ExitStack

    import concourse.bass as bass
    import concourse.tile as tile
    from concourse import bacc, mybir
    f32 = mybir.dt.float32
    bf16 = mybir.dt.bfloat16

    nc = bacc.Bacc(
        "TRN2",
        target_bir_lowering=False,
        debug=False,
        enable_asserts=False,
        num_devices=NCORES,
    )
    q_ap = nc.dram_tensor("query", [BL, N, C], f32, kind="ExternalInput").ap()
    k_ap = nc.dram_tensor("key", [BL, S, C], f32, kind="ExternalInput").ap()
    wq_ap = nc.dram_tensor("Wq", [C, C], f32, kind="ExternalInput").ap()
    wk_ap = nc.dram_tensor("Wk", [C, C], f32, kind="ExternalInput").ap()
    out_ap = nc.dram_tensor("out", [BL, N, C], f32, kind="ExternalOutput").ap()
    out2_ap = nc.dram_tensor("out_style", [BL, N, C], f32, kind="ExternalOutput").ap()

    NT = S // 128   # 32 s-tiles
    NCK = C // 128  # c_in chunks
    NM = C // 128   # c_out chunks

    with tile.TileContext(nc) as tc, ExitStack() as ctx:
        const = ctx.enter_context(tc.tile_pool(name="const", bufs=1))
        wq_bf = const.tile([128, NCK * C], bf16)
        wk_bf = const.tile([128, NCK * C], bf16)
        nc.gpsimd.dma_start(
            wq_bf[:].rearrange("p (k c) -> p k c", k=NCK),
            wq_ap.rearrange("(k p) c -> p k c", k=NCK),
        )
        nc.gpsimd.dma_start(
            wk_bf[:].rearrange("p (k c) -> p k c", k=NCK),
            wk_ap.rearrange("(k p) c -> p k c", k=NCK),
        )

        # SBUF pools
        kb_pool = ctx.enter_context(tc.tile_pool(name="kb", bufs=2))
        ktp_pool = ctx.enter_context(tc.tile_pool(name="ktp", bufs=2))
        ktin_pool = ctx.enter_context(tc.tile_pool(name="ktin", bufs=1))
        qpool = ctx.enter_context(tc.tile_pool(name="qpool", bufs=2))
        epool = ctx.enter_context(tc.tile_pool(name="epool", bufs=3))
        spool = ctx.enter_context(tc.tile_pool(name="spool", bufs=3))
        opool = ctx.enter_context(tc.tile_pool(name="opool", bufs=2))
        # DRAM bounce scratch
        dpool = ctx.enter_context(tc.tile_pool(name="dram", bufs=2, space="DRAM"))

        # PSUM pools: kprj 1 + scA 2 + scB 2 + oacc 3 = 8 banks
        kprj_pool = ctx.enter_context(tc.tile_pool(name="kprj", bufs=1, space="PSUM"))
        scA_pool = ctx.enter_context(tc.tile_pool(name="scA", bufs=1, space="PSUM"))
        scB_pool = ctx.enter_context(tc.tile_pool(name="scB", bufs=1, space="PSUM"))
        oacc_pool = ctx.enter_context(tc.tile_pool(name="oacc", bufs=1, space="PSUM"))

        for b in range(BL):
            # ---------- q path: bounce-transpose + projection ----------
            qsc = dpool.tile([N, C], bf16, tag="qsc")
            nc.gpsimd.dma_start(qsc[:], q_ap[b])  # DRAM->DRAM cast f32->bf16
            qt_sb = qpool.tile([128, NCK * N], bf16, tag="qt")
            for ck in range(NCK):
                nc.sync.dma_start(
                    qt_sb[:, ck * N : (ck + 1) * N],
                    qsc[:, ck * 128 : (ck + 1) * 128],
                    transpose=True,
                )
            qtp = qpool.tile([128, NM * N], bf16, tag="qtp")
            for m in range(NM):
                pq = kprj_pool.tile([128, 512], f32, tag="kprj")
                for k in range(NCK):
                    nc.tensor.matmul(
                        pq[:, :N],
                        lhsT=wq_bf[:, k * C + m * 128 : k * C + (m + 1) * 128],
                        rhs=qt_sb[:, k * N : (k + 1) * N],
                        start=(k == 0),
                        stop=(k == NCK - 1),
                    )
                nc.vector.tensor_copy(qtp[:, m * N : (m + 1) * N], pq[:, :N])

            # ---------- k path ----------
            # kb65: [s-part, (t, h, 1+64)] with ones col per head
            kb = kb_pool.tile([128, NT * H * 65], bf16, tag="kb")
            kb4 = kb[:].rearrange("p (t h x) -> p t h x", t=NT, h=H)
            nc.vector.memset(kb4[:, :, :, 0:1], 1.0)
            # (filled per s-block below, from the bf16 scratch)
            # bf16 scratch copy of key straight from DRAM (cast during DMA),
            # chunked by 512-row blocks; then xbar-transpose-read keyT per
            # block so the k-projection pipelines behind the DMA chain.
            ksc = dpool.tile([S, C], bf16, tag="ksc")
            for sb in range(S // 512):
                nc.gpsimd.dma_start(
                    ksc[sb * 512 : (sb + 1) * 512, :],
                    k_ap[b, sb * 512 : (sb + 1) * 512, :],
                )
            ktin = ktin_pool.tile([128, NCK * S], bf16, tag="ktin")
            ktp = ktp_pool.tile([128, NM * S], bf16, tag="ktp")

            # ---------- attention (interleaved with the k-path below) ----------
            oacc = oacc_pool.tile([128, 1536], f32, tag="oacc")

            def do_tile(t):
                # scores: tile A = even heads (row group 0), B = odd (rg 64)
                scA = scA_pool.tile([128, 1024], f32, tag="scA")
                scB = scB_pool.tile([128, 1024], f32, tag="scB")
                for i in range(4):
                    for sc, h in ((scA, HORD[i]), (scB, HORD[4 + i])):
                        m, hp = h // 2, (h % 2) * 64
                        nc.tensor.matmul(
                            sc[:, i * N : (i + 1) * N],
                            lhsT=ktp[
                                hp : hp + 64, m * S + t * 128 : m * S + t * 128 + 128
                            ],
                            rhs=qtp[hp : hp + 64, m * N : (m + 1) * N],
                            start=True,
                            stop=True,
                        )
                # exp: A in one FD-1024 instr; B as 4 FD-256 instrs with accum
                et = epool.tile([128, 2 * 1024], bf16, tag="et")
                den = spool.tile([128, 8], f32, tag="den")
                nc.scalar.activation(
                    et[:, 0:1024],
                    scA[:],
                    mybir.ActivationFunctionType.Exp,
                    scale=SCALE,
                )
                for i in range(4):
                    nc.scalar.activation(
                        et[:, 1024 + i * N : 1024 + (i + 1) * N],
                        scB[:, i * N : (i + 1) * N],
                        mybir.ActivationFunctionType.Exp,
                        scale=SCALE,
                        accum_out=den[:, 4 + i : 5 + i],
                    )
                # denominators for A heads: one grouped reduce
                nc.vector.tensor_reduce(
                    den[:, 0:4],
                    et[:, 0:1024].rearrange("p (g n) -> p g n", g=4),
                    mybir.AxisListType.X,
                    mybir.AluOpType.add,
                )
                rt = spool.tile([128, 8], f32, tag="rt")
                nc.vector.reciprocal(rt[:], den[:])
                # vaug[idx] = [r | v*r] from the ones-embedded kb slice
                vaug = spool.tile([128, 8 * 65], bf16, tag="vaug")
                for idx in range(8):
                    h = HORD[idx]
                    nc.vector.tensor_scalar_mul(
                        vaug[:, idx * 65 : (idx + 1) * 65],
                        kb[:, (t * H + h) * 65 : (t * H + h) * 65 + 65],
                        rt[:, idx : idx + 1],
                    )
                # out matmuls: one accumulation group per PSUM bank
                crit = (
                    tc.tile_critical()
                    if (t == 0 or t == NT - 1)
                    else contextlib.nullcontext()
                )
                with crit:
                    for idx in range(8):
                        for ncn in range(2):
                            g = idx * 2 + ncn
                            nc.tensor.matmul(
                                oacc[:, _g_col(g) : _g_col(g) + 65],
                                lhsT=et[
                                    :, idx * N + ncn * 128 : idx * N + ncn * 128 + 128
                                ],
                                rhs=vaug[:, idx * 65 : (idx + 1) * 65],
                                start=(t == 0 and g in (0, 7, 14)),
                                stop=(t == NT - 1 and g in (6, 13, 15)),
                                skip_group_check=True,
                            )

            for sb in range(S // 512):
                for h in range(H):
                    nc.sync.dma_start(
                        kb4[:, 4 * sb : 4 * sb + 4, h, 1:65],
                        ksc[sb * 512 : (sb + 1) * 512, h * HD : (h + 1) * HD]
                        .rearrange("(t p) c -> p t c", t=4),
                    )
                for ck in range(NCK):
                    nc.sync.dma_start(
                        ktin[:, ck * S + sb * 512 : ck * S + (sb + 1) * 512],
                        ksc[sb * 512 : (sb + 1) * 512, ck * 128 : (ck + 1) * 128],
                        transpose=True,
                    )
                for m in range(NM):
                    pk = kprj_pool.tile([128, 512], f32, tag="kprj")
                    for k in range(NCK):
                        nc.tensor.matmul(
                            pk[:],
                            lhsT=wk_bf[:, k * C + m * 128 : k * C + (m + 1) * 128],
                            rhs=ktin[:, k * S + sb * 512 : k * S + (sb + 1) * 512],
                            start=(k == 0),
                            stop=(k == NCK - 1),
                        )
                    nc.vector.tensor_copy(
                        ktp[:, m * S + sb * 512 : m * S + (sb + 1) * 512], pk[:]
                    )
                for t in range(4 * sb, 4 * sb + 4):
                    do_tile(t)

            # ---------- epilogue ----------
            dm = spool.tile([128, 16], f32, tag="dm")
            nc.vector.tensor_scalar_max(
                dm[:, 0:7],
                oacc[:, 0:455].rearrange("p (g x) -> p g x", g=7)[:, :, 0:1],
                1.0,
            )
            nc.vector.tensor_scalar_max(
                dm[:, 7:14],
                oacc[:, 512:967].rearrange("p (g x) -> p g x", g=7)[:, :, 0:1],
                1.0,
            )
            nc.vector.tensor_scalar_max(
                dm[:, 14:16],
                oacc[:, 1024:1154].rearrange("p (g x) -> p g x", g=2)[:, :, 0:1],
                1.0,
            )
            rdiv = spool.tile([128, 16], f32, tag="rdiv")
            nc.vector.reciprocal(rdiv[:], dm[:])
            for ncn in range(2):
                osb = opool.tile([128, C], f32, tag="osb")
                for idx in range(8):
                    h = HORD[idx]
                    g = idx * 2 + ncn
                    nc.vector.tensor_scalar_mul(
                        osb[:, h * HD : (h + 1) * HD],
                        oacc[:, _g_col(g) + 1 : _g_col(g) + 65],
                        rdiv[:, g : g + 1],
                    )
                nc.sync.dma_start(out_ap[b, ncn * 128 : (ncn + 1) * 128, :], osb[:])
                nc.sync.dma_start(out2_ap[b, ncn * 128 : (ncn + 1) * 128, :], osb[:])

    nc.compile()
    return nc


def _get_nc():
    with _lock:
        if "nc" not in _cache:
            _cache["nc"] = _build()
        return _cache["nc"]


def kernel(query, key, Wq, Wk):
    from concourse.bass_utils import run_bass_kernel_spmd

    nc = _get_nc()
    query = np.ascontiguousarray(query, dtype=np.float32)
    key = np.ascontiguousarray(key, dtype=np.float32)
    Wq = np.ascontiguousarray(Wq, dtype=np.float32)
    Wk = np.ascontiguousarray(Wk, dtype=np.float32)
    in_maps = [
        {
            "query": query[c * BL : (c + 1) * BL],
            "key": key[c * BL : (c + 1) * BL],
            "Wq": Wq,
            "Wk": Wk,
        }
        for c in range(NCORES)
    ]
    res = run_bass_kernel_spmd(nc, in_maps, core_ids=list(range(NCORES)))
    out = np.concatenate([r["out"] for r in res.results], axis=0)
    out_style = np.concatenate([r["out_style"] for r in res.results], axis=0)
    return out, out_style
